# revision 18
# baseline (speedup 1.0000x reference)
"""AMS loss kernel for Trainium2, data-parallel over 8 NeuronCores.

Reference computation (per row r of logits [N, C], target t_r):
    num_r   = logits[r, t_r]
    denom_r = exp(num_r) + (sum_j exp(logits[r, j])) * e^M - exp(num_r) * e^M
    L_r     = num_r - log(denom_r + EPS)
    out     = -mean_r(L_r)

Sharding: rows (N=16384) split evenly across 8 cores (2048 rows each).
Each core streams its shard through SBUF in 16 tiles of [128 rows, 10000
cols]; the scalar engine computes exp(x + M) with a fused row-sum
(accum_out), the vector engine gathers the target logit via a fused
(iota == target) * logits multiply-accumulate, and the epilogue reduces
to a single partial sum which the host combines across cores.

Raw Bass (no Tile): the fused-reduce DVE instruction (S2S2D2_STT) and the
HWDGE DMA instruction have too few embedded sync-wait slots for Tile's
auto-generated dependency waits, so synchronization is explicit
standalone wait_ge instructions per engine.
"""

import sys
import numpy as np

for _p in ("/opt/trn_rl_repo",):
    if _p not in sys.path:
        sys.path.insert(0, _p)

N_TOTAL = 16384
C = 10000
N_CORES = 8
ROWS = N_TOTAL // N_CORES        # 2048 rows per core
P = 128                          # partitions
TILES = ROWS // P                # 16 row-tiles per core
M = 0.4
EPS = 1e-10
NBUF = 2                         # big-tile double buffering

PROFILE = False                  # set True (e.g. by test.py) to capture NTFF profile
LAST_RESULT = None               # BassKernelResults of the last run (for profiling)

_CACHE = {}


def _build_nc():
    from contextlib import ExitStack

    import concourse.bass as bass
    import concourse.mybir as mybir

    F32 = mybir.dt.float32
    BF16 = mybir.dt.bfloat16
    I32 = mybir.dt.int32
    Alu = mybir.AluOpType
    Act = mybir.ActivationFunctionType

    EXP_M = float(np.exp(np.float32(M)))

    nc = bass.Bass()
    logits = nc.declare_dram_parameter("logits", [ROWS, C], F32, isOutput=False)
    tgt = nc.declare_dram_parameter("tgt", [P, TILES], I32, isOutput=False)
    out = nc.declare_dram_parameter("out", [1, 1], F32, isOutput=True)

    logits_t = logits.rearrange("(n p) c -> n p c", p=P)

    with ExitStack() as ctx:
        en_ctx = ctx.enter_context
        tb = [
            en_ctx(nc.sbuf_tensor(f"tb{i}", [P, C], F32)) for i in range(NBUF)
        ]
        iota_f = en_ctx(nc.sbuf_tensor([P, C], F32))
        g_dve = en_ctx(nc.sbuf_tensor([P, C], BF16))   # unused elementwise out
        g_act = en_ctx(nc.sbuf_tensor([P, C], BF16))   # unused elementwise out
        bias_m = en_ctx(nc.sbuf_tensor([P, 1], F32))
        bias_eps = en_ctx(nc.sbuf_tensor([P, 1], F32))
        tgt_i = en_ctx(nc.sbuf_tensor([P, TILES], I32))
        tgt_f = en_ctx(nc.sbuf_tensor([P, TILES], F32))
        summ = en_ctx(nc.sbuf_tensor([P, TILES], F32))
        num = en_ctx(nc.sbuf_tensor([P, TILES], F32))
        en = en_ctx(nc.sbuf_tensor([P, TILES], F32))
        denom = en_ctx(nc.sbuf_tensor([P, TILES], F32))
        lnd = en_ctx(nc.sbuf_tensor([P, TILES], F32))
        lg = en_ctx(nc.sbuf_tensor([P, TILES], F32))
        partial = en_ctx(nc.sbuf_tensor([P, 1], F32))
        red = en_ctx(nc.sbuf_tensor([1, 1], F32))
        res = en_ctx(nc.sbuf_tensor([1, 1], F32))

        # One DMA-completion semaphore per buffer slot: increments on one sem
        # are serialized by the buffer-free handshake (compute j done before
        # DMA j+NBUF issues), so completions can never race within a sem.
        tgt_sem = en_ctx(nc.semaphore("tgt_sem"))
        buf_sem = [en_ctx(nc.semaphore(f"buf_sem{i}")) for i in range(NBUF)]
        out_sem = en_ctx(nc.semaphore("out_sem"))
        v_sem = en_ctx(nc.semaphore("v_sem"))
        a_sem = en_ctx(nc.semaphore("a_sem"))
        p_sem = en_ctx(nc.semaphore("p_sem"))

        block = en_ctx(nc.Block())

        # buf_sem[j % NBUF] == 16*(j//NBUF + 1)  <=>  tile j loaded
        # v_sem: tgt_f copy -> 1 ; gather j -> j+2 ; denom -> T+2 ; lg -> T+3
        # a_sem: exp j -> j+1 ; en -> T+1 ; lnd -> T+2 ; res -> T+3
        # p_sem: consts (iota/biases) -> 1 ; red -> 2

        @block.sync
        def _(sync):
            sync.dma_start(out=tgt_i[:], in_=tgt[:]).then_inc(tgt_sem, 16)
            for j in range(TILES):
                if j >= NBUF:
                    sync.wait_ge(v_sem, j - NBUF + 2)
                    sync.wait_ge(a_sem, j - NBUF + 1)
                sync.dma_start(out=tb[j % NBUF][:], in_=logits_t[j]).then_inc(
                    buf_sem[j % NBUF], 16
                )
            sync.wait_ge(a_sem, TILES + 3)
            sync.dma_start(out=out[:], in_=res[:]).then_inc(out_sem, 16)

        @block.gpsimd
        def _(gpsimd):
            gpsimd.memset(bias_m[:], M)
            gpsimd.memset(bias_eps[:], EPS)
            gpsimd.iota(
                iota_f[:],
                pattern=[[1, C]],
                base=0,
                channel_multiplier=0,
                allow_small_or_imprecise_dtypes=True,
            ).then_inc(p_sem, 1)
            gpsimd.wait_ge(v_sem, TILES + 3)
            gpsimd.tensor_reduce(
                red[:], partial[:], axis=mybir.AxisListType.C, op=Alu.add
            ).then_inc(p_sem, 1)

        @block.vector
        def _(vector):
            vector.wait_ge(tgt_sem, 16)
            vector.tensor_copy(tgt_f[:], tgt_i[:]).then_inc(v_sem, 1)
            vector.wait_ge(p_sem, 1)
            for j in range(TILES):
                # own-engine wait: prior DVE op retired (engine is pipelined)
                vector.wait_ge(v_sem, j + 1)
                vector.wait_ge(buf_sem[j % NBUF], 16 * (j // NBUF + 1))
                vector.scalar_tensor_tensor(
                    out=g_dve[:],
                    in0=iota_f[:],
                    scalar=tgt_f[:, j : j + 1],
                    in1=tb[j % NBUF][:],
                    op0=Alu.is_equal,
                    op1=Alu.mult,
                    accum_out=num[:, j : j + 1],
                ).then_inc(v_sem, 1)
            vector.wait_ge(a_sem, TILES + 1)
            vector.scalar_tensor_tensor(
                out=denom[:],
                in0=en[:],
                scalar=1.0 - EXP_M,
                in1=summ[:],
                op0=Alu.mult,
                op1=Alu.add,
            ).then_inc(v_sem, 1)
            vector.wait_ge(a_sem, TILES + 2)
            vector.wait_ge(v_sem, TILES + 2)
            vector.scalar_tensor_tensor(
                out=lg[:],
                in0=num[:],
                scalar=1.0,
                in1=lnd[:],
                op0=Alu.mult,
                op1=Alu.subtract,
                accum_out=partial[:],
            ).then_inc(v_sem, 1)

        @block.scalar
        def _(scalar):
            scalar.wait_ge(p_sem, 1)
            for j in range(TILES):
                scalar.wait_ge(a_sem, j)
                scalar.wait_ge(buf_sem[j % NBUF], 16 * (j // NBUF + 1))
                scalar.activation(
                    out=g_act[:],
                    in_=tb[j % NBUF][:],
                    func=Act.Exp,
                    bias=bias_m[:],
                    scale=1.0,
                    accum_out=summ[:, j : j + 1],
                ).then_inc(a_sem, 1)
            scalar.wait_ge(v_sem, TILES + 1)
            scalar.activation(out=en[:], in_=num[:], func=Act.Exp).then_inc(a_sem, 1)
            scalar.wait_ge(v_sem, TILES + 2)
            scalar.activation(
                out=lnd[:], in_=denom[:], func=Act.Ln, bias=bias_eps[:]
            ).then_inc(a_sem, 1)
            scalar.wait_ge(p_sem, 2)
            scalar.mul(res[:], red[:], -1.0 / N_TOTAL).then_inc(a_sem, 1)

    return nc


def _get_nc():
    if "nc" not in _CACHE:
        _CACHE["nc"] = _build_nc()
    return _CACHE["nc"]


def kernel(logits, targets):
    global LAST_RESULT
    from concourse.bass_utils import run_bass_kernel_spmd

    logits = np.ascontiguousarray(np.asarray(logits), dtype=np.float32)
    targets = np.asarray(targets).astype(np.int32)
    assert logits.shape == (N_TOTAL, C), logits.shape
    assert targets.shape == (N_TOTAL,), targets.shape

    in_maps = []
    for k in range(N_CORES):
        lo, hi = k * ROWS, (k + 1) * ROWS
        shard = logits[lo:hi]
        # tile j, partition p holds row j*128+p -> tgt[p, j] = targets[lo + j*128 + p]
        tgt_shard = np.ascontiguousarray(targets[lo:hi].reshape(TILES, P).T)
        in_maps.append({"logits": shard, "tgt": tgt_shard})

    nc = _get_nc()
    result = run_bass_kernel_spmd(
        nc, in_maps, core_ids=list(range(N_CORES)), trace=PROFILE
    )
    LAST_RESULT = result
    total = np.float64(0.0)
    for r in result.results:
        total += np.float64(r["out"].reshape(-1)[0])
    return np.float32(total)


# revision 20
# speedup vs baseline: 1.0196x; 1.0196x over previous
"""AMS loss kernel for Trainium2, data-parallel over 8 NeuronCores.

Reference computation (per row r of logits [N, C], target t_r):
    num_r   = logits[r, t_r]
    denom_r = exp(num_r) + (sum_j exp(logits[r, j])) * e^M - exp(num_r) * e^M
    L_r     = num_r - log(denom_r + EPS)
    out     = -mean_r(L_r)

Sharding: rows (N=16384) split evenly across 8 cores (2048 rows each).
Each core streams its shard through SBUF in 16 tiles of [128 rows, 10000
cols]; the scalar engine computes exp(x + M) with a fused row-sum
(accum_out), the vector engine gathers the target logit via a fused
(iota == target) * logits multiply-accumulate, and the epilogue reduces
to a single partial sum which the host combines across cores.

Raw Bass (no Tile): the fused-reduce DVE instruction (S2S2D2_STT) and the
HWDGE DMA instruction have too few embedded sync-wait slots for Tile's
auto-generated dependency waits, so synchronization is explicit
standalone wait_ge instructions per engine.

Schedule notes (from NTFF profile):
 - The logits stream runs at HBM line rate (~387 GB/s) on the SP HWDGE
   queue; everything else must stay off that queue and under its period.
 - The targets DMA goes through gpsimd's SWDGE so its 128 tiny strided
   descriptors don't delay the first logits tile.
 - The LAST tile is split into column chunks so only one small chunk's
   gather (~2.7us instead of 10.6us) remains exposed after the stream.
"""

import sys
import numpy as np

for _p in ("/opt/trn_rl_repo",):
    if _p not in sys.path:
        sys.path.insert(0, _p)

N_TOTAL = 16384
C = 10000
N_CORES = 8
ROWS = N_TOTAL // N_CORES        # 2048 rows per core
P = 128                          # partitions
TILES = ROWS // P                # 16 row-tiles per core
M = 0.4
EPS = 1e-10
NBUF = 3                         # big-tile buffering
NCHUNK = 4                       # column chunks for the last tile
CCOLS = C // NCHUNK              # 2500
NREG = TILES - 1                 # regular (unsplit) tiles

PROFILE = False                  # set True (e.g. by test.py) to capture NTFF profile
LAST_RESULT = None               # BassKernelResults of the last run (for profiling)

_CACHE = {}


def _build_nc():
    from contextlib import ExitStack

    import concourse.bass as bass
    import concourse.mybir as mybir

    F32 = mybir.dt.float32
    BF16 = mybir.dt.bfloat16
    I32 = mybir.dt.int32
    Alu = mybir.AluOpType
    Act = mybir.ActivationFunctionType

    EXP_M = float(np.exp(np.float32(M)))

    nc = bass.Bass()
    logits = nc.declare_dram_parameter("logits", [ROWS, C], F32, isOutput=False)
    tgt = nc.declare_dram_parameter("tgt", [P, TILES], I32, isOutput=False)
    out = nc.declare_dram_parameter("out", [1, 1], F32, isOutput=True)

    logits_t = logits.rearrange("(n p) c -> n p c", p=P)

    with ExitStack() as ctx:
        en_ctx = ctx.enter_context
        tb = [
            en_ctx(nc.sbuf_tensor(f"tb{i}", [P, C], F32)) for i in range(NBUF)
        ]
        iota_f = en_ctx(nc.sbuf_tensor([P, C], F32))
        g_dve = en_ctx(nc.sbuf_tensor([P, C], BF16))   # unused elementwise out
        g_act = en_ctx(nc.sbuf_tensor([P, C], BF16))   # unused elementwise out
        bias_m = en_ctx(nc.sbuf_tensor([P, 1], F32))
        bias_eps = en_ctx(nc.sbuf_tensor([P, 1], F32))
        tgt_i = en_ctx(nc.sbuf_tensor([P, TILES], I32))
        tgt_f = en_ctx(nc.sbuf_tensor([P, TILES], F32))
        summ = en_ctx(nc.sbuf_tensor([P, TILES], F32))
        num = en_ctx(nc.sbuf_tensor([P, TILES], F32))
        num_x = en_ctx(nc.sbuf_tensor([P, NCHUNK], F32))
        summ_x = en_ctx(nc.sbuf_tensor([P, NCHUNK], F32))
        en = en_ctx(nc.sbuf_tensor([P, TILES], F32))
        denom = en_ctx(nc.sbuf_tensor([P, TILES], F32))
        lnd = en_ctx(nc.sbuf_tensor([P, TILES], F32))
        lg = en_ctx(nc.sbuf_tensor([P, TILES], F32))
        partial = en_ctx(nc.sbuf_tensor([P, 1], F32))
        red = en_ctx(nc.sbuf_tensor([1, 1], F32))
        res = en_ctx(nc.sbuf_tensor([1, 1], F32))

        tgt_sem = en_ctx(nc.semaphore("tgt_sem"))
        buf_sem = [en_ctx(nc.semaphore(f"buf_sem{i}")) for i in range(NBUF)]
        chk_sem = [en_ctx(nc.semaphore(f"chk_sem{i}")) for i in range(NCHUNK)]
        out_sem = en_ctx(nc.semaphore("out_sem"))
        v_sem = en_ctx(nc.semaphore("v_sem"))
        a_sem = en_ctx(nc.semaphore("a_sem"))
        b_sem = en_ctx(nc.semaphore("b_sem"))
        p_sem = en_ctx(nc.semaphore("p_sem"))

        block = en_ctx(nc.Block())

        # Semaphore timelines:
        #  buf_sem[j % NBUF] == 16*(j//NBUF + 1)  <=>  regular tile j loaded
        #  chk_sem[c] == 16                       <=>  last-tile chunk c loaded
        #  tgt_sem == 16                          <=>  targets loaded (SWDGE)
        #  b_sem: DVE bias memsets -> 1
        #  p_sem: iota -> 1 ; tgt cast -> 2 ; red -> 3
        #  v_sem: gather j -> j+1 (j<NREG) ; chunk gathers -> NREG+1..NREG+4 ;
        #         num_x reduce -> 20 ; summ_x reduce -> 21 ; denom -> 22 ; lg -> 23
        #  a_sem: exp j -> j+1 (j<NREG) ; chunk exps -> NREG+1..NREG+4 ;
        #         en -> 20 ; lnd -> 21 ; res -> 22

        V_GATHER_DONE = NREG + NCHUNK          # 19
        V_NUMX = V_GATHER_DONE + 1             # 20
        V_SUMX = V_NUMX + 1                    # 21
        V_DENOM = V_SUMX + 1                   # 22
        V_LG = V_DENOM + 1                     # 23
        A_EXP_DONE = NREG + NCHUNK             # 19
        A_EN = A_EXP_DONE + 1                  # 20
        A_LND = A_EN + 1                       # 21
        A_RES = A_LND + 1                      # 22

        @block.sync
        def _(sync):
            for j in range(NREG):
                if j >= NBUF:
                    sync.wait_ge(v_sem, j - NBUF + 1)
                    sync.wait_ge(a_sem, j - NBUF + 1)
                sync.dma_start(out=tb[j % NBUF][:], in_=logits_t[j]).then_inc(
                    buf_sem[j % NBUF], 16
                )
            # last tile reuses the slot of tile TILES-1-NBUF
            sync.wait_ge(v_sem, TILES - NBUF)
            sync.wait_ge(a_sem, TILES - NBUF)
            last = logits_t[TILES - 1]
            lslot = (TILES - 1) % NBUF
            for c in range(NCHUNK):
                cs = slice(c * CCOLS, (c + 1) * CCOLS)
                sync.dma_start(out=tb[lslot][:, cs], in_=last[:, cs]).then_inc(
                    chk_sem[c], 16
                )
            sync.wait_ge(a_sem, A_RES)
            sync.dma_start(out=out[:], in_=res[:]).then_inc(out_sem, 16)

        @block.gpsimd
        def _(gpsimd):
            gpsimd.dma_start(out=tgt_i[:], in_=tgt[:]).then_inc(tgt_sem, 16)
            gpsimd.iota(
                iota_f[:],
                pattern=[[1, C]],
                base=0,
                channel_multiplier=0,
                allow_small_or_imprecise_dtypes=True,
            ).then_inc(p_sem, 1)
            gpsimd.wait_ge(tgt_sem, 16)
            gpsimd.tensor_copy(tgt_f[:], tgt_i[:]).then_inc(p_sem, 1)
            gpsimd.wait_ge(v_sem, V_LG)
            gpsimd.tensor_reduce(
                red[:], partial[:], axis=mybir.AxisListType.C, op=Alu.add
            ).then_inc(p_sem, 1)

        @block.vector
        def _(vector):
            vector.memset(bias_m[:], M)
            vector.memset(bias_eps[:], EPS).then_inc(b_sem, 1)
            vector.wait_ge(p_sem, 2)   # iota + targets cast
            for j in range(NREG):
                vector.wait_ge(v_sem, j)
                vector.wait_ge(buf_sem[j % NBUF], 16 * (j // NBUF + 1))
                vector.scalar_tensor_tensor(
                    out=g_dve[:],
                    in0=iota_f[:],
                    scalar=tgt_f[:, j : j + 1],
                    in1=tb[j % NBUF][:],
                    op0=Alu.is_equal,
                    op1=Alu.mult,
                    accum_out=num[:, j : j + 1],
                ).then_inc(v_sem, 1)
            lslot = (TILES - 1) % NBUF
            for c in range(NCHUNK):
                cs = slice(c * CCOLS, (c + 1) * CCOLS)
                vector.wait_ge(v_sem, NREG + c)
                vector.wait_ge(chk_sem[c], 16)
                vector.scalar_tensor_tensor(
                    out=g_dve[:, 0:CCOLS],
                    in0=iota_f[:, cs],
                    scalar=tgt_f[:, TILES - 1 : TILES],
                    in1=tb[lslot][:, cs],
                    op0=Alu.is_equal,
                    op1=Alu.mult,
                    accum_out=num_x[:, c : c + 1],
                ).then_inc(v_sem, 1)
            # fold the chunk partials into column TILES-1
            vector.wait_ge(v_sem, V_GATHER_DONE)
            vector.tensor_reduce(
                num[:, TILES - 1 : TILES], num_x[:], axis=mybir.AxisListType.X,
                op=Alu.add,
            ).then_inc(v_sem, 1)
            vector.wait_ge(a_sem, A_EXP_DONE)
            vector.wait_ge(v_sem, V_NUMX)
            vector.tensor_reduce(
                summ[:, TILES - 1 : TILES], summ_x[:], axis=mybir.AxisListType.X,
                op=Alu.add,
            ).then_inc(v_sem, 1)
            vector.wait_ge(a_sem, A_EN)
            vector.wait_ge(v_sem, V_SUMX)
            vector.scalar_tensor_tensor(
                out=denom[:],
                in0=en[:],
                scalar=1.0 - EXP_M,
                in1=summ[:],
                op0=Alu.mult,
                op1=Alu.add,
            ).then_inc(v_sem, 1)
            vector.wait_ge(a_sem, A_LND)
            vector.wait_ge(v_sem, V_DENOM)
            vector.scalar_tensor_tensor(
                out=lg[:],
                in0=num[:],
                scalar=1.0,
                in1=lnd[:],
                op0=Alu.mult,
                op1=Alu.subtract,
                accum_out=partial[:],
            ).then_inc(v_sem, 1)

        @block.scalar
        def _(scalar):
            scalar.wait_ge(b_sem, 1)
            for j in range(NREG):
                scalar.wait_ge(a_sem, j)
                scalar.wait_ge(buf_sem[j % NBUF], 16 * (j // NBUF + 1))
                scalar.activation(
                    out=g_act[:],
                    in_=tb[j % NBUF][:],
                    func=Act.Exp,
                    bias=bias_m[:],
                    scale=1.0,
                    accum_out=summ[:, j : j + 1],
                ).then_inc(a_sem, 1)
            lslot = (TILES - 1) % NBUF
            for c in range(NCHUNK):
                cs = slice(c * CCOLS, (c + 1) * CCOLS)
                scalar.wait_ge(a_sem, NREG + c)
                scalar.wait_ge(chk_sem[c], 16)
                scalar.activation(
                    out=g_act[:, 0:CCOLS],
                    in_=tb[lslot][:, cs],
                    func=Act.Exp,
                    bias=bias_m[:],
                    scale=1.0,
                    accum_out=summ_x[:, c : c + 1],
                ).then_inc(a_sem, 1)
            scalar.wait_ge(v_sem, V_NUMX)
            scalar.activation(out=en[:], in_=num[:], func=Act.Exp).then_inc(a_sem, 1)
            scalar.wait_ge(v_sem, V_DENOM)
            scalar.activation(
                out=lnd[:], in_=denom[:], func=Act.Ln, bias=bias_eps[:]
            ).then_inc(a_sem, 1)
            scalar.wait_ge(p_sem, 3)
            scalar.mul(res[:], red[:], -1.0 / N_TOTAL).then_inc(a_sem, 1)

    return nc


def _get_nc():
    if "nc" not in _CACHE:
        _CACHE["nc"] = _build_nc()
    return _CACHE["nc"]


def kernel(logits, targets):
    global LAST_RESULT
    from concourse.bass_utils import run_bass_kernel_spmd

    logits = np.ascontiguousarray(np.asarray(logits), dtype=np.float32)
    targets = np.asarray(targets).astype(np.int32)
    assert logits.shape == (N_TOTAL, C), logits.shape
    assert targets.shape == (N_TOTAL,), targets.shape

    in_maps = []
    for k in range(N_CORES):
        lo, hi = k * ROWS, (k + 1) * ROWS
        shard = logits[lo:hi]
        # tile j, partition p holds row j*128+p -> tgt[p, j] = targets[lo + j*128 + p]
        tgt_shard = np.ascontiguousarray(targets[lo:hi].reshape(TILES, P).T)
        in_maps.append({"logits": shard, "tgt": tgt_shard})

    nc = _get_nc()
    result = run_bass_kernel_spmd(
        nc, in_maps, core_ids=list(range(N_CORES)), trace=PROFILE
    )
    LAST_RESULT = result
    total = np.float64(0.0)
    for r in result.results:
        total += np.float64(r["out"].reshape(-1)[0])
    return np.float32(total)
